# revision 11
# baseline (speedup 1.0000x reference)
"""AiVad (retrieval 1-NN + GMM) Trainium2 kernel.

Strategy (8 NeuronCores, SPMD):
  - Memory banks (pose_bank [65536,34], feature_bank [65536,512]) are sharded
    along the bank dim: 8192 rows per core. Queries + GMM params replicated.
  - Per core, for each score:
      appearance: s[n,m] = 2 q.b  (bf16 matmul, 4 K-tiles of 128)
                  then fused DVE tensor_tensor_reduce adds (C - |b|^2) in f32
                  and max-reduces over m straight out of PSUM.
      pose:       one K=104 bf16 matmul per tile: 3-way bf16 error-split of
                  q.b (hi*hi + hi*lo + lo*hi) plus two bias rows carrying
                  (C - |b|^2) split into bf16 hi/lo -> near-fp32 accuracy.
      velocity:   GMM log-likelihood via fp32 matmul z = (x-mu) @ (L/sqrt(2))
                  + ACT Square/Exp/Ln + DVE reduces (exact logsumexp, K=5).
  - Each core writes a [128, 48] partial: per query-tile max of
    s = 2q.b - |b|^2 + C for app/pose, and the full velocity loglik.
  - Host: max over cores, d2 = q2 + C - smax, sqrt, min-max normalize.
"""

import math

import numpy as np
import ml_dtypes

BF16 = ml_dtypes.bfloat16
F32 = np.float32

P = 128          # partitions
N = 2048         # queries
QT = N // P      # 16 query tiles
DA = 512         # appearance dim
DP = 34          # pose dim
DV = 8           # velocity dim
KG = 5           # gmm components
M = 65536        # bank rows
NCORES = 8
MSH = M // NCORES  # 8192 bank rows per core
KA = DA // P     # 4 appearance k-tiles
G = 8            # m-groups per core
GW = MSH // G    # 1024 group width (2 psum banks)
NCHUNK = GW // 512  # 2 matmul chunks per group
KP = 3 * DP + 2  # 104 pose k rows (hi*hi, hi*lo, lo*hi, bias hi, bias lo)
LN2PI = float(np.log(2.0 * np.pi))

_cache: dict = {}


def _split_multi_waits(bir_json: bytes) -> bytes:
    """Split instructions with >1 attached sem-waits into single-wait
    EventSemaphore instructions (this walrus build rejects multi-wait
    encodings with 'Too many sync wait commands'). Waits here are
    monotonic sem-ge waits, so sequential waiting is equivalent."""
    import json as _json

    j = _json.loads(bir_json)
    cnt = [0]

    def fix_block(blk):
        out = []
        for inst in blk.get("instructions", []):
            si = inst.get("sync_info")
            waits = (si or {}).get("on_wait") or []
            if len(waits) > 1:
                for w in waits[:-1]:
                    cnt[0] += 1
                    out.append(
                        {
                            "debug": inst.get("debug", 0),
                            "engine": inst["engine"],
                            "ins": [],
                            "outs": [],
                            "name": f"swait{cnt[0]}_{inst['name']}",
                            "opcode": "EventSemaphore",
                            "sync_info": {"on_update": [], "on_wait": [w]},
                        }
                    )
                si["on_wait"] = [waits[-1]]
            out.append(inst)
        blk["instructions"] = out
        for sb in blk.get("blocks", []):
            fix_block(sb)

    for fn in j["functions"]:
        for blk in fn.get("blocks", []):
            fix_block(blk)
    return _json.dumps(j).encode()


def _install_wait_split_patch():
    import concourse.bass_utils as bu
    import concourse.bass2jax as bj

    if getattr(bu, "_wait_split_patched", False):
        return
    orig = bu.compile_bir_kernel

    def patched(bir_json, tmpdir, neff_name="file.neff"):
        return orig(_split_multi_waits(bytes(bir_json)), tmpdir, neff_name=neff_name)

    bu.compile_bir_kernel = patched
    bj.compile_bir_kernel = patched
    bu._wait_split_patched = True


def _build_bass():
    import concourse.bass as bass
    import concourse.mybir as mybir
    import concourse.tile as tile
    from contextlib import ExitStack

    dt = mybir.dt
    AX = mybir.AxisListType
    ALU = mybir.AluOpType
    AF = mybir.ActivationFunctionType

    nc = bass.Bass()

    qkt_app = nc.declare_dram_parameter("qkt_app", [DA, N], dt.bfloat16, isOutput=False)
    bkt_app = nc.declare_dram_parameter("bkt_app", [DA, MSH], dt.bfloat16, isOutput=False)
    aug_app = nc.declare_dram_parameter("aug_app", [2, MSH], dt.bfloat16, isOutput=False)
    ones2 = nc.declare_dram_parameter("ones2", [2, P], dt.bfloat16, isOutput=False)
    qkt_pose = nc.declare_dram_parameter("qkt_pose", [KP, N], dt.bfloat16, isOutput=False)
    bkt_pose = nc.declare_dram_parameter("bkt_pose", [KP, MSH], dt.bfloat16, isOutput=False)
    gmm_lhs = nc.declare_dram_parameter("gmm_lhs", [DV + 1, N], dt.float32, isOutput=False)
    gmm_rhs = nc.declare_dram_parameter("gmm_rhs", [DV + 1, KG * DV], dt.float32, isOutput=False)
    gmm_c = nc.declare_dram_parameter("gmm_c", [P, QT * KG], dt.float32, isOutput=False)
    out_ext = nc.declare_dram_parameter("out_part", [P, 3 * QT], dt.float32, isOutput=True)

    with tile.TileContext(nc) as tc, ExitStack() as ctx:
        const = ctx.enter_context(tc.tile_pool(name="const", bufs=1))
        psum = ctx.enter_context(tc.tile_pool(name="psum", bufs=2, space="PSUM"))

        # --- query-side + gmm constants (loaded once) ---
        sb_qa = []
        for k in range(KA):
            t = const.tile([P, N], dt.bfloat16, tag=f"qa{k}")
            nc.sync.dma_start(t[:], qkt_app[k * P:(k + 1) * P, :])
            sb_qa.append(t)
        sb_qp = const.tile([KP, N], dt.bfloat16, tag="qp")
        nc.sync.dma_start(sb_qp[:], qkt_pose[:, :])
        sb_ones = const.tile([2, P], dt.bfloat16, tag="ones2")
        nc.sync.dma_start(sb_ones[:], ones2[:, :])
        sb_glhs = const.tile([DV + 1, N], dt.float32, tag="glhs")
        nc.sync.dma_start(sb_glhs[:], gmm_lhs[:, :])
        sb_grhs = const.tile([DV + 1, KG * DV], dt.float32, tag="grhs")
        nc.sync.dma_start(sb_grhs[:], gmm_rhs[:, :])
        sb_gc = const.tile([P, QT * KG], dt.float32, tag="gc")
        nc.sync.dma_start(sb_gc[:], gmm_c[:, :])

        # --- bank-side tiles, one set per m-group ---
        sb_ba = {}
        sb_cr = {}
        sb_bp = {}
        for g in range(G):
            gs = slice(g * GW, (g + 1) * GW)
            for k in range(KA):
                t = const.tile([P, GW], dt.bfloat16, tag=f"ba{g}_{k}")
                nc.sync.dma_start(t[:], bkt_app[k * P:(k + 1) * P, gs])
                sb_ba[g, k] = t
            t = const.tile([2, GW], dt.bfloat16, tag=f"aug{g}")
            nc.sync.dma_start(t[:], aug_app[:, gs])
            sb_cr[g] = t
            t = const.tile([KP, GW], dt.bfloat16, tag=f"bp{g}")
            nc.sync.dma_start(t[:], bkt_pose[:, gs])
            sb_bp[g] = t

        acc_a = const.tile([P, QT * G], dt.float32, tag="acca")
        acc_p = const.tile([P, QT * G], dt.float32, tag="accp")
        outp = const.tile([P, 3 * QT], dt.float32, tag="outp")

        # --- main loop: appearance + pose, m-group outer for DMA overlap ---
        for g in range(G):
            for q in range(QT):
                qs = slice(q * P, (q + 1) * P)
                col = q * G + g
                pa = psum.tile([P, GW], dt.float32, tag="pa")
                for k in range(KA):
                    for c in range(NCHUNK):
                        cs = slice(c * 512, (c + 1) * 512)
                        nc.tensor.matmul(
                            pa[:, cs],
                            lhsT=sb_qa[k][:, qs],
                            rhs=sb_ba[g, k][:, cs],
                            start=(k == 0),
                            stop=False,
                        )
                # bias rows: accumulate (C - b2) via K=2 ones matmul
                for c in range(NCHUNK):
                    cs = slice(c * 512, (c + 1) * 512)
                    nc.tensor.matmul(
                        pa[:, cs],
                        lhsT=sb_ones[:],
                        rhs=sb_cr[g][:, cs],
                        start=False,
                        stop=True,
                    )
                nc.vector.tensor_reduce(
                    out=acc_a[:, col:col + 1], in_=pa[:], axis=AX.X, op=ALU.max
                )

                pp = psum.tile([P, GW], dt.float32, tag="pp")
                for c in range(NCHUNK):
                    cs = slice(c * 512, (c + 1) * 512)
                    nc.tensor.matmul(
                        pp[:, cs],
                        lhsT=sb_qp[:, qs],
                        rhs=sb_bp[g][:, cs],
                        start=True,
                        stop=True,
                    )
                nc.vector.tensor_reduce(
                    out=acc_p[:, col:col + 1], in_=pp[:], axis=AX.X, op=ALU.max
                )

        # --- velocity GMM ---
        z2 = const.tile([P, QT * KG * DV], dt.float32, tag="z2")
        for q in range(QT):
            qs = slice(q * P, (q + 1) * P)
            pg = psum.tile([P, GW], dt.float32, tag="pa")
            nc.tensor.matmul(
                pg[:, : KG * DV],
                lhsT=sb_glhs[:, qs],
                rhs=sb_grhs[:],
                start=True,
                stop=True,
            )
            # z2 = z'^2   (maha/2 = sum_e z'^2 since L was scaled by 1/sqrt2)
            nc.scalar.activation(
                out=z2[:, q * KG * DV:(q + 1) * KG * DV],
                in_=pg[:, : KG * DV],
                func=AF.Square,
            )
        maha2 = const.tile([P, QT * KG], dt.float32, tag="maha2")
        nc.vector.tensor_reduce(
            out=maha2[:],
            in_=z2[:].rearrange("p (x e) -> p x e", e=DV),
            axis=AX.X,
            op=ALU.add,
        )
        targ = const.tile([P, QT * KG], dt.float32, tag="targ")
        nc.vector.tensor_tensor(targ[:], sb_gc[:], maha2[:], ALU.subtract)
        earg = const.tile([P, QT * KG], dt.float32, tag="earg")
        nc.scalar.activation(out=earg[:], in_=targ[:], func=AF.Exp)
        ssum = const.tile([P, QT], dt.float32, tag="ssum")
        nc.vector.tensor_reduce(
            out=ssum[:],
            in_=earg[:].rearrange("p (t k) -> p t k", k=KG),
            axis=AX.X,
            op=ALU.add,
        )
        nc.scalar.activation(out=outp[:, 2 * QT:3 * QT], in_=ssum[:], func=AF.Ln)

        # --- final max over m-groups ---
        nc.vector.tensor_reduce(
            out=outp[:, 0:QT],
            in_=acc_a[:].rearrange("p (q g) -> p q g", g=G),
            axis=AX.X,
            op=ALU.max,
        )
        nc.vector.tensor_reduce(
            out=outp[:, QT:2 * QT],
            in_=acc_p[:].rearrange("p (q g) -> p q g", g=G),
            axis=AX.X,
            op=ALU.max,
        )

        nc.sync.dma_start(out_ext[:, :], outp[:])

    return nc


def _get_nc():
    if "nc" not in _cache:
        _cache["nc"] = _build_bass()
    return _cache["nc"]


def _split_bf16(x):
    """x (f32/f64) -> (hi, lo) bf16 with hi+lo ~= x."""
    hi = x.astype(BF16)
    lo = (x - hi.astype(np.float64)).astype(BF16)
    return hi, lo


def prepare(inputs):
    """Host-side shard + layout prep. Returns (in_maps, host_ctx)."""
    velocity = np.asarray(inputs["velocity"], np.float32)
    pose = np.asarray(inputs["pose"], np.float32)
    appearance = np.asarray(inputs["appearance"], np.float32)
    pose_bank = np.asarray(inputs["pose_bank"], np.float32)
    feature_bank = np.asarray(inputs["feature_bank"], np.float32)
    gmm_means = np.asarray(inputs["gmm_means"], np.float64)
    gmm_prec_chol = np.asarray(inputs["gmm_prec_chol"], np.float64)
    gmm_log_weights = np.asarray(inputs["gmm_log_weights"], np.float64)

    # ---- replicated query-side tensors ----
    qkt_app = np.ascontiguousarray((2.0 * appearance).T).astype(BF16)  # [512, 2048]

    a = (2.0 * pose).astype(np.float64)  # [2048, 34]
    ahi, alo = _split_bf16(a)
    qkt_pose = np.empty((KP, N), BF16)
    qkt_pose[0:DP] = ahi.T
    qkt_pose[DP:2 * DP] = ahi.T
    qkt_pose[2 * DP:3 * DP] = alo.T
    qkt_pose[3 * DP:] = np.ones((2, N), BF16)

    # ---- gmm constants ----
    pcs = gmm_prec_chol / math.sqrt(2.0)  # [5, 8, 8]
    gmm_rhs = np.empty((DV + 1, KG * DV), np.float32)
    for k in range(KG):
        gmm_rhs[0:DV, k * DV:(k + 1) * DV] = pcs[k]
        gmm_rhs[DV, k * DV:(k + 1) * DV] = -(gmm_means[k] @ pcs[k])
    logdet = np.log(np.diagonal(gmm_prec_chol, axis1=1, axis2=2)).sum(1)  # [5]
    c5 = gmm_log_weights + logdet - 0.5 * DV * LN2PI
    gmm_c = np.broadcast_to(
        np.tile(c5.astype(np.float32), QT), (P, QT * KG)
    ).copy()
    gmm_lhs = np.empty((DV + 1, N), np.float32)
    gmm_lhs[0:DV] = velocity.T
    gmm_lhs[DV] = 1.0

    # ---- bank norms / shift constants (global, f64) ----
    b2_app = (feature_bank.astype(np.float64) ** 2).sum(1)  # [65536]
    b2_pose = (pose_bank.astype(np.float64) ** 2).sum(1)
    C_app = float(b2_app.mean())
    C_pose = float(b2_pose.mean())

    in_maps = []
    for ci in range(NCORES):
        sl = slice(ci * MSH, (ci + 1) * MSH)
        B = feature_bank[sl]  # [8192, 512]
        bkt_app = np.ascontiguousarray(B.T).astype(BF16)
        va = C_app - b2_app[sl]  # [8192] f64
        vahi, valo = _split_bf16(va)
        aug_app = np.stack([vahi, valo])  # [2, 8192] bf16

        Bp = pose_bank[sl].astype(np.float64)  # [8192, 34]
        bhi, blo = _split_bf16(Bp)
        vp = C_pose - b2_pose[sl]  # [8192]
        vhi, vlo = _split_bf16(vp)
        bkt_pose = np.empty((KP, MSH), BF16)
        bkt_pose[0:DP] = bhi.T
        bkt_pose[DP:2 * DP] = blo.T
        bkt_pose[2 * DP:3 * DP] = bhi.T
        bkt_pose[3 * DP] = vhi
        bkt_pose[3 * DP + 1] = vlo

        in_maps.append(
            {
                "qkt_app": qkt_app,
                "bkt_app": bkt_app,
                "aug_app": aug_app,
                "ones2": np.ones((2, P), BF16),
                "qkt_pose": qkt_pose,
                "bkt_pose": bkt_pose,
                "gmm_lhs": gmm_lhs,
                "gmm_rhs": gmm_rhs,
                "gmm_c": gmm_c,
            }
        )

    q2_app = (appearance.astype(np.float64) ** 2).sum(1)  # [2048]
    q2_pose = (pose.astype(np.float64) ** 2).sum(1)
    host_ctx = {
        "q2_app": q2_app,
        "q2_pose": q2_pose,
        "C_app": C_app,
        "C_pose": C_pose,
        "vel_min": float(np.asarray(inputs["vel_min"]).reshape(-1)[0]),
        "vel_max": float(np.asarray(inputs["vel_max"]).reshape(-1)[0]),
        "pose_min": float(np.asarray(inputs["pose_min"]).reshape(-1)[0]),
        "pose_max": float(np.asarray(inputs["pose_max"]).reshape(-1)[0]),
        "feat_min": float(np.asarray(inputs["feat_min"]).reshape(-1)[0]),
        "feat_max": float(np.asarray(inputs["feat_max"]).reshape(-1)[0]),
    }
    return in_maps, host_ctx


def combine(results, host_ctx):
    """Gather per-core [128, 48] partials -> full [3, 2048] output."""
    parts = np.stack([np.asarray(r["out_part"], np.float64) for r in results])
    # [8, 128, 48]; columns: 0:16 app smax, 16:32 pose smax, 32:48 vel loglik
    smax_app = parts[:, :, 0:QT].max(0).T.reshape(N)       # n = t*128 + p
    smax_pose = parts[:, :, QT:2 * QT].max(0).T.reshape(N)
    loglik = parts[0, :, 2 * QT:3 * QT].T.reshape(N)

    d2a = host_ctx["q2_app"] + host_ctx["C_app"] - smax_app
    d2p = host_ctx["q2_pose"] + host_ctx["C_pose"] - smax_pose
    dist_a = np.sqrt(np.maximum(d2a, 1e-12))
    dist_p = np.sqrt(np.maximum(d2p, 1e-12))

    vel_s = (-loglik - host_ctx["vel_min"]) / (host_ctx["vel_max"] - host_ctx["vel_min"])
    pose_s = (dist_p - host_ctx["pose_min"]) / (host_ctx["pose_max"] - host_ctx["pose_min"])
    app_s = (dist_a - host_ctx["feat_min"]) / (host_ctx["feat_max"] - host_ctx["feat_min"])
    return np.stack([vel_s, pose_s, app_s]).astype(np.float32)


def run_device(in_maps, trace=False, **kwargs):
    from concourse.bass_utils import run_bass_kernel_spmd

    _install_wait_split_patch()
    return run_bass_kernel_spmd(
        _get_nc(), in_maps, list(range(NCORES)), trace=trace, **kwargs
    )


def kernel(**inputs) -> np.ndarray:
    in_maps, host_ctx = prepare(inputs)
    res = run_device(in_maps)
    return combine(res.results, host_ctx)


# revision 34
# speedup vs baseline: 1.2059x; 1.2059x over previous
"""AiVad (retrieval 1-NN + GMM) Trainium2 kernel.

Strategy (8 NeuronCores, SPMD):
  - Memory banks (pose_bank [65536,34], feature_bank [65536,512]) are sharded
    along the bank dim: 8192 rows per core. Queries + GMM params replicated.
  - appearance: s[n,m] = 2 q.b via bf16 matmul (4 K-tiles of 128). The
    per-row bias (C - |b|^2) is NOT accumulated in the matmul: the shard's
    rows are sorted by |b|^2 on the host and laid out so that the 8 rows a
    SIMD lane sees across the 8 m-groups have nearly equal bias; the scan
    then max-reduces raw s per lane (ACT copies PSUM->bf16 with a per-query
    centering bias, DVE pairwise-maxes at 2x bf16 rate), and the per-lane
    bias is added once at the end (f32). Host undoes the centering exactly.
  - pose: one K=104 bf16 matmul per tile: 3-way bf16 error-split of q.b
    (hi*hi + hi*lo + lo*hi) plus two bias rows carrying (C - |b|^2) split
    into bf16 hi/lo -> near-fp32 accuracy; DVE reduce_max from PSUM.
  - velocity: GMM log-likelihood via fp32 matmul z = (x-mu) @ (L/sqrt(2))
    + ACT Square/Exp/Ln + DVE reduces (exact logsumexp, K=5).
  - Each core writes a [128, 48] partial; host maxes over cores, rebuilds
    d2 = q2 + C - smax, takes sqrt and min-max normalizes.
"""

import math

import numpy as np
import ml_dtypes

BF16 = ml_dtypes.bfloat16

P = 128          # partitions
N = 2048         # queries
QT = N // P      # 16 query tiles
DA = 512         # appearance dim
DP = 34          # pose dim
DV = 8           # velocity dim
KG = 5           # gmm components
M = 65536        # bank rows
NCORES = 8
MSH = M // NCORES  # 8192 bank rows per core
KA = DA // P     # 4 appearance k-tiles
G = 8            # m-groups per core
GW = MSH // G    # 1024 group width (2 psum banks)
NCHUNK = GW // 512  # 2 matmul chunks per group
KP = 3 * DP + 2  # 104 pose k rows (hi*hi, hi*lo, lo*hi, bias hi, bias lo)
LN2PI = float(np.log(2.0 * np.pi))

_cache: dict = {}


def _split_multi_waits(bir_json: bytes) -> bytes:
    """Split instructions with >1 attached sem-waits into single-wait
    EventSemaphore instructions (this walrus build rejects multi-wait
    encodings with 'Too many sync wait commands'). Waits here are
    monotonic sem-ge waits, so sequential waiting is equivalent."""
    import json as _json

    j = _json.loads(bir_json)
    cnt = [0]

    def fix_block(blk):
        out = []
        for inst in blk.get("instructions", []):
            si = inst.get("sync_info")
            waits = (si or {}).get("on_wait") or []
            if len(waits) > 1:
                for w in waits[:-1]:
                    cnt[0] += 1
                    out.append(
                        {
                            "debug": inst.get("debug", 0),
                            "engine": inst["engine"],
                            "ins": [],
                            "outs": [],
                            "name": f"swait{cnt[0]}_{inst['name']}",
                            "opcode": "EventSemaphore",
                            "sync_info": {"on_update": [], "on_wait": [w]},
                        }
                    )
                si["on_wait"] = [waits[-1]]
            out.append(inst)
        blk["instructions"] = out
        for sb in blk.get("blocks", []):
            fix_block(sb)

    for fn in j["functions"]:
        for blk in fn.get("blocks", []):
            fix_block(blk)
    return _json.dumps(j).encode()


def _install_wait_split_patch():
    import concourse.bass_utils as bu
    import concourse.bass2jax as bj

    if getattr(bu, "_wait_split_patched", False):
        return
    orig = bu.compile_bir_kernel

    def patched(bir_json, tmpdir, neff_name="file.neff"):
        return orig(_split_multi_waits(bytes(bir_json)), tmpdir, neff_name=neff_name)

    bu.compile_bir_kernel = patched
    bj.compile_bir_kernel = patched
    bu._wait_split_patched = True


def _build_bass():
    import concourse.bass as bass
    import concourse.mybir as mybir
    import concourse.tile as tile
    from contextlib import ExitStack

    dt = mybir.dt
    AX = mybir.AxisListType
    ALU = mybir.AluOpType
    AF = mybir.ActivationFunctionType

    nc = bass.Bass()

    qkt_app = nc.declare_dram_parameter("qkt_app", [DA, N], dt.bfloat16, isOutput=False)
    bkt_app = nc.declare_dram_parameter("bkt_app", [DA, MSH], dt.bfloat16, isOutput=False)
    sbias = nc.declare_dram_parameter("sbias", [P, QT], dt.float32, isOutput=False)
    aug_app = nc.declare_dram_parameter("aug_app", [2, 2 * GW], dt.bfloat16, isOutput=False)
    ones2 = nc.declare_dram_parameter("ones2", [2, P], dt.bfloat16, isOutput=False)
    qkt_pose = nc.declare_dram_parameter("qkt_pose", [KP, N], dt.bfloat16, isOutput=False)
    bkt_pose = nc.declare_dram_parameter("bkt_pose", [KP, MSH], dt.bfloat16, isOutput=False)
    gmm_lhs = nc.declare_dram_parameter("gmm_lhs", [DV + 1, N], dt.float32, isOutput=False)
    gmm_rhs = nc.declare_dram_parameter("gmm_rhs", [DV + 1, KG * DV], dt.float32, isOutput=False)
    gmm_c = nc.declare_dram_parameter("gmm_c", [P, QT * KG], dt.float32, isOutput=False)
    out_ext = nc.declare_dram_parameter("out_part", [P, 4 * QT], dt.float32, isOutput=True)
    runs_ext = nc.declare_dram_parameter("runs", [QT, P, GW], dt.bfloat16, isOutput=True)

    with tile.TileContext(nc) as tc, ExitStack() as ctx:
        const = ctx.enter_context(tc.tile_pool(name="const", bufs=1))
        work = ctx.enter_context(tc.tile_pool(name="work", bufs=3))
        psum = ctx.enter_context(tc.tile_pool(name="psum", bufs=2, space="PSUM"))

        # --- tiles; DMAs issued in first-use order so PE starts early ---
        sb_qa = []
        sb_ba = {}
        sb_bp = {}

        def load_group(g):
            gs = slice(g * GW, (g + 1) * GW)
            for k in range(KA):
                t = const.tile([P, GW], dt.bfloat16, tag=f"ba{g}_{k}", name=f"ba{g}_{k}")
                nc.sync.dma_start(t[:], bkt_app[k * P:(k + 1) * P, gs])
                sb_ba[g, k] = t
            t = const.tile([KP, GW], dt.bfloat16, tag=f"bp{g}", name=f"bp{g}")
            nc.sync.dma_start(t[:], bkt_pose[:, gs])
            sb_bp[g] = t

        # interleave query k-tiles with group-0 bank k-tiles: the (g0,q0)
        # matmul chain needs (qa[k], ba[0,k]) pairs in order.
        for k in range(KA):
            t = const.tile([P, N], dt.bfloat16, tag=f"qa{k}", name=f"qa{k}")
            nc.sync.dma_start(t[:], qkt_app[k * P:(k + 1) * P, :])
            sb_qa.append(t)
            bt = const.tile([P, GW], dt.bfloat16, tag=f"ba0_{k}", name=f"ba0_{k}")
            nc.sync.dma_start(bt[:], bkt_app[k * P:(k + 1) * P, 0:GW])
            sb_ba[0, k] = bt
        bt = const.tile([KP, GW], dt.bfloat16, tag="bp0", name="bp0")
        nc.sync.dma_start(bt[:], bkt_pose[:, 0:GW])
        sb_bp[0] = bt
        sb_ones = const.tile([2, P], dt.bfloat16, tag="ones2")
        nc.sync.dma_start(sb_ones[:], ones2[:, :])
        sb_aug = const.tile([2, 2 * GW], dt.bfloat16, tag="aug")
        nc.sync.dma_start(sb_aug[:], aug_app[:, :])
        sb_qp = const.tile([KP, N], dt.bfloat16, tag="qp")
        nc.sync.dma_start(sb_qp[:], qkt_pose[:, :])
        load_group(1)
        sb_sb = const.tile([P, QT], dt.float32, tag="sbias")
        nc.sync.dma_start(sb_sb[:], sbias[:, :])
        sb_glhs = const.tile([DV + 1, N], dt.float32, tag="glhs")
        nc.sync.dma_start(sb_glhs[:], gmm_lhs[:, :])
        sb_grhs = const.tile([DV + 1, KG * DV], dt.float32, tag="grhs")
        nc.sync.dma_start(sb_grhs[:], gmm_rhs[:, :])
        sb_gc = const.tile([P, QT * KG], dt.float32, tag="gc")
        nc.sync.dma_start(sb_gc[:], gmm_c[:, :])
        for g in range(2, G):
            load_group(g)

        acc_p = const.tile([P, QT * G], dt.float32, tag="accp")
        outp = const.tile([P, 4 * QT], dt.float32, tag="outp")
        run_a = [
            const.tile([P, GW], dt.bfloat16, tag=f"run{q}", name=f"run{q}")
            for q in range(QT)
        ]

        # --- velocity GMM pieces, interleaved into the main loop ---
        z2 = const.tile([P, QT * KG * DV], dt.float32, tag="z2")

        def gmm_mm(q):
            qs = slice(q * P, (q + 1) * P)
            pg = psum.tile([P, GW], dt.float32, tag="pa", name="pg")
            nc.tensor.matmul(
                pg[:, : KG * DV],
                lhsT=sb_glhs[:, qs],
                rhs=sb_grhs[:],
                start=True,
                stop=True,
            )
            nc.scalar.activation(
                out=z2[:, q * KG * DV:(q + 1) * KG * DV],
                in_=pg[:, : KG * DV],
                func=AF.Square,
            )

        def gmm_chain():
            maha2 = const.tile([P, QT * KG], dt.float32, tag="maha2")
            nc.vector.tensor_reduce(
                out=maha2[:],
                in_=z2[:].rearrange("p (x e) -> p x e", e=DV),
                axis=AX.X,
                op=ALU.add,
            )
            targ = const.tile([P, QT * KG], dt.float32, tag="targ")
            nc.vector.tensor_tensor(targ[:], sb_gc[:], maha2[:], ALU.subtract)
            earg = const.tile([P, QT * KG], dt.float32, tag="earg")
            nc.scalar.activation(out=earg[:], in_=targ[:], func=AF.Exp)
            ssum = const.tile([P, QT], dt.float32, tag="ssum")
            nc.vector.tensor_reduce(
                out=ssum[:],
                in_=earg[:].rearrange("p (t k) -> p t k", k=KG),
                axis=AX.X,
                op=ALU.add,
            )
            nc.scalar.activation(out=outp[:, QT:2 * QT], in_=ssum[:], func=AF.Ln)

        # --- main loop: appearance + pose, m-group outer for DMA overlap ---
        for g in range(G):
            for q in range(QT):
                qs = slice(q * P, (q + 1) * P)
                col = q * G + g
                pa = psum.tile([P, GW], dt.float32, tag="pa")
                for k in range(KA):
                    for c in range(NCHUNK):
                        cs = slice(c * 512, (c + 1) * 512)
                        nc.tensor.matmul(
                            pa[:, cs],
                            lhsT=sb_qa[k][:, qs],
                            rhs=sb_ba[g, k][:, cs],
                            start=(k == 0),
                            stop=(k == KA - 1) and g not in (0, G - 1),
                        )
                if g in (0, G - 1):
                    # exact tail group: accumulate (C - b2) rows, f32 reduce
                    sg = 0 if g == 0 else 1
                    for c in range(NCHUNK):
                        cs = slice(c * 512, (c + 1) * 512)
                        nc.tensor.matmul(
                            pa[:, cs],
                            lhsT=sb_ones[:],
                            rhs=sb_aug[:, sg * GW + c * 512: sg * GW + (c + 1) * 512],
                            start=False,
                            stop=True,
                        )
                    nc.vector.tensor_reduce(
                        out=outp[:, (2 + sg) * QT + q:(2 + sg) * QT + q + 1],
                        in_=pa[:], axis=AX.X, op=ALU.max,
                    )
                else:
                    # middle groups: centered bf16 copy; running max over groups
                    if g == 1:
                        nc.scalar.activation(
                            out=run_a[q][:], in_=pa[:], func=AF.Identity,
                            bias=sb_sb[:, q:q + 1],
                        )
                    else:
                        scr = work.tile([P, GW], dt.bfloat16, tag="scr")
                        nc.scalar.activation(
                            out=scr[:], in_=pa[:], func=AF.Identity,
                            bias=sb_sb[:, q:q + 1],
                        )
                        nc.vector.tensor_tensor(
                            run_a[q][:], run_a[q][:], scr[:], ALU.max
                        )

                pp = psum.tile([P, GW], dt.float32, tag="pp")
                for c in range(NCHUNK):
                    cs = slice(c * 512, (c + 1) * 512)
                    nc.tensor.matmul(
                        pp[:, cs],
                        lhsT=sb_qp[:, qs],
                        rhs=sb_bp[g][:, cs],
                        start=True,
                        stop=True,
                    )
                nc.vector.tensor_reduce(
                    out=acc_p[:, col:col + 1], in_=pp[:], axis=AX.X, op=ALU.max
                )
                if g == 2:
                    gmm_mm(q)
                if g == 3 and q == 0:
                    gmm_chain()
                if g == G - 2:
                    # run_a[q] complete (middle groups are 1..G-2): ship raw
                    # lane maxes; host adds the lane bias and reduces.
                    nc.sync.dma_start(runs_ext[q], run_a[q][:])

        # --- pose final ---
        nc.vector.tensor_reduce(
            out=outp[:, 0:QT],
            in_=acc_p[:].rearrange("p (q g) -> p q g", g=G),
            axis=AX.X,
            op=ALU.max,
        )

        nc.sync.dma_start(out_ext[:, :], outp[:])

    # Hoist matmul sem-waits onto the paired ldweights so the wait overlaps
    # the previous matmul's drain (same pass Bacc.compile runs).
    import bass_rust as _br

    _br.move_matmul_waits_to_ldweights(nc.m)
    return nc


def _get_nc():
    if "nc" not in _cache:
        _cache["nc"] = _build_bass()
    return _cache["nc"]


def _split_bf16(x):
    """x (f32/f64) -> (hi, lo) bf16 with hi+lo ~= x."""
    hi = x.astype(BF16)
    lo = (x - hi.astype(np.float64)).astype(BF16)
    return hi, lo


def prepare(inputs):
    """Host-side shard + layout prep. Returns (in_maps, host_ctx)."""
    velocity = np.asarray(inputs["velocity"], np.float32)
    pose = np.asarray(inputs["pose"], np.float32)
    appearance = np.asarray(inputs["appearance"], np.float32)
    pose_bank = np.asarray(inputs["pose_bank"], np.float32)
    feature_bank = np.asarray(inputs["feature_bank"], np.float32)
    gmm_means = np.asarray(inputs["gmm_means"], np.float64)
    gmm_prec_chol = np.asarray(inputs["gmm_prec_chol"], np.float64)
    gmm_log_weights = np.asarray(inputs["gmm_log_weights"], np.float64)

    # ---- replicated query-side tensors ----
    qkt_app = np.ascontiguousarray((2.0 * appearance).T).astype(BF16)  # [512, 2048]

    a = (2.0 * pose).astype(np.float64)  # [2048, 34]
    ahi, alo = _split_bf16(a)
    qkt_pose = np.empty((KP, N), BF16)
    qkt_pose[0:DP] = ahi.T
    qkt_pose[DP:2 * DP] = ahi.T
    qkt_pose[2 * DP:3 * DP] = alo.T
    qkt_pose[3 * DP:] = np.ones((2, N), BF16)

    # ---- gmm constants ----
    pcs = gmm_prec_chol / math.sqrt(2.0)  # [5, 8, 8]
    gmm_rhs = np.empty((DV + 1, KG * DV), np.float32)
    for k in range(KG):
        gmm_rhs[0:DV, k * DV:(k + 1) * DV] = pcs[k]
        gmm_rhs[DV, k * DV:(k + 1) * DV] = -(gmm_means[k] @ pcs[k])
    logdet = np.log(np.diagonal(gmm_prec_chol, axis1=1, axis2=2)).sum(1)  # [5]
    c5 = gmm_log_weights + logdet - 0.5 * DV * LN2PI
    gmm_c = np.broadcast_to(
        np.tile(c5.astype(np.float32), QT), (P, QT * KG)
    ).copy()
    gmm_lhs = np.empty((DV + 1, N), np.float32)
    gmm_lhs[0:DV] = velocity.T
    gmm_lhs[DV] = 1.0

    # ---- bank norms / shift constants (global, f64) ----
    b2_app = (feature_bank.astype(np.float64) ** 2).sum(1)  # [65536]
    b2_pose = (pose_bank.astype(np.float64) ** 2).sum(1)
    C_app = float(b2_app.mean())
    C_pose = float(b2_pose.mean())
    q2_app = (appearance.astype(np.float64) ** 2).sum(1)  # [2048]
    q2_pose = (pose.astype(np.float64) ** 2).sum(1)

    # per-query centering bias for the bf16 scan: the winning value of
    # max_m 2 q.b is ~ 2*sqrt(q2 * 2 ln M) for near-gaussian data. Only
    # scan precision depends on this; the host subtracts it back exactly.
    sbias_n = (-2.0 * np.sqrt(q2_app * (2.0 * math.log(M)))).astype(np.float32)
    sbias = np.ascontiguousarray(sbias_n.reshape(QT, P).T)  # [128, 16]

    in_maps = []
    cls = []
    for ci in range(NCORES):
        sl = slice(ci * MSH, (ci + 1) * MSH)
        B = feature_bank[sl]  # [8192, 512]
        b2s = b2_app[sl]
        # Sort rows by |b|^2. Tail groups 0 (bottom GW ranks) and G-1 (top
        # GW ranks) get exact per-row bias via aug matmul rows; the middle
        # 6 groups are laid out so lane j sees 6 consecutive mid-distribution
        # ranks (near-constant bias, handled once per lane at the end).
        order = np.argsort(b2s, kind="stable")  # rank -> row
        nmid = G - 2
        rank_of_col = np.empty((G, GW), np.int64)
        rank_of_col[0] = np.arange(GW)
        rank_of_col[G - 1] = MSH - GW + np.arange(GW)
        for gg in range(1, G - 1):
            rank_of_col[gg] = GW + np.arange(GW) * nmid + (gg - 1)
        cols = order[rank_of_col.reshape(MSH)]
        bkt_app = np.ascontiguousarray(B[cols].T).astype(BF16)  # [512, 8192]
        cval = C_app - b2s[order]  # [8192] by rank
        cl = cval[GW:MSH - GW].reshape(GW, nmid).mean(1)
        cls.append(cl)
        aug = np.empty((2, 2 * GW))
        aug[0, 0:GW] = cval[rank_of_col[0]]
        aug[0, GW:] = cval[rank_of_col[G - 1]]
        ahi2, alo2 = _split_bf16(aug[0])
        aug_app = np.empty((2, 2 * GW), BF16)
        aug_app[0] = ahi2
        aug_app[1] = alo2

        Bp = pose_bank[sl].astype(np.float64)  # [8192, 34]
        bhi, blo = _split_bf16(Bp)
        vp = C_pose - b2_pose[sl]  # [8192]
        vhi, vlo = _split_bf16(vp)
        bkt_pose = np.empty((KP, MSH), BF16)
        bkt_pose[0:DP] = bhi.T
        bkt_pose[DP:2 * DP] = blo.T
        bkt_pose[2 * DP:3 * DP] = bhi.T
        bkt_pose[3 * DP] = vhi
        bkt_pose[3 * DP + 1] = vlo

        in_maps.append(
            {
                "qkt_app": qkt_app,
                "bkt_app": bkt_app,
                "sbias": sbias,
                "aug_app": aug_app,
                "ones2": np.ones((2, P), BF16),
                "qkt_pose": qkt_pose,
                "bkt_pose": bkt_pose,
                "gmm_lhs": gmm_lhs,
                "gmm_rhs": gmm_rhs,
                "gmm_c": gmm_c,
            }
        )

    host_ctx = {
        "cls": cls,
        "q2_app": q2_app,
        "q2_pose": q2_pose,
        "C_app": C_app,
        "C_pose": C_pose,
        "sbias_n": sbias_n.astype(np.float64),
        "vel_min": float(np.asarray(inputs["vel_min"]).reshape(-1)[0]),
        "vel_max": float(np.asarray(inputs["vel_max"]).reshape(-1)[0]),
        "pose_min": float(np.asarray(inputs["pose_min"]).reshape(-1)[0]),
        "pose_max": float(np.asarray(inputs["pose_max"]).reshape(-1)[0]),
        "feat_min": float(np.asarray(inputs["feat_min"]).reshape(-1)[0]),
        "feat_max": float(np.asarray(inputs["feat_max"]).reshape(-1)[0]),
    }
    return in_maps, host_ctx


def combine(results, host_ctx):
    """Gather per-core [128, 48] partials -> full [3, 2048] output."""
    parts = np.stack([np.asarray(r["out_part"], np.float64) for r in results])
    # out_part cols: 0:16 pose smax, 16:32 vel loglik, 32:48 app tail-low
    # smax, 48:64 app tail-high smax. "runs" carries the app middle-lane
    # running maxes (+bias); the lane bias is added here.
    lane_vals = []
    for ci, r in enumerate(results):
        runs = np.asarray(r["runs"], np.float64)  # [QT, P, GW]
        cl = host_ctx["cls"][ci]
        lane_vals.append((runs + cl[None, None, :]).max(-1).reshape(N))
    lane_term = np.max(lane_vals, axis=0) - host_ctx["sbias_n"]
    sp = np.maximum(parts[:, :, 2 * QT:3 * QT], parts[:, :, 3 * QT:4 * QT])
    sp_term = sp.max(0).T.reshape(N)
    smax_app = np.maximum(lane_term, sp_term)
    smax_pose = parts[:, :, 0:QT].max(0).T.reshape(N)
    loglik = parts[0, :, QT:2 * QT].T.reshape(N)

    d2a = host_ctx["q2_app"] + host_ctx["C_app"] - smax_app
    d2p = host_ctx["q2_pose"] + host_ctx["C_pose"] - smax_pose
    dist_a = np.sqrt(np.maximum(d2a, 1e-12))
    dist_p = np.sqrt(np.maximum(d2p, 1e-12))

    vel_s = (-loglik - host_ctx["vel_min"]) / (host_ctx["vel_max"] - host_ctx["vel_min"])
    pose_s = (dist_p - host_ctx["pose_min"]) / (host_ctx["pose_max"] - host_ctx["pose_min"])
    app_s = (dist_a - host_ctx["feat_min"]) / (host_ctx["feat_max"] - host_ctx["feat_min"])
    return np.stack([vel_s, pose_s, app_s]).astype(np.float32)


def run_device(in_maps, trace=False, **kwargs):
    from concourse.bass_utils import run_bass_kernel_spmd

    _install_wait_split_patch()
    return run_bass_kernel_spmd(
        _get_nc(), in_maps, list(range(NCORES)), trace=trace, **kwargs
    )


def kernel(**inputs) -> np.ndarray:
    in_maps, host_ctx = prepare(inputs)
    res = run_device(in_maps)
    return combine(res.results, host_ctx)
